# revision 11
# baseline (speedup 1.0000x reference)
"""L2-bounded LTI cell (SSM scan) as a truncated convolution on TRN2.

Math: per batch b the reference computes
    x_{t+1} = x_t @ A.T + u_t @ B.T
    y_t     = x_t @ C.T + u_t @ D.T
with outputs x_seq[t] = x_t (pre-update state) and y_seq[t] = y_t.

K = K_raw / (||K_raw||_2 + 0.002) is a strict contraction, so
||A^m||_2 decays ~0.47x per step and the scan is a causal convolution
    x_t = x0 @ At^t + sum_{m<M} u_{t-1-m} @ G_m,   G_m = Bt @ At^m
truncated at M taps (M=6: structured trunc err ~2e-3, well under the
2e-2 gate).

Precision (validated in simacc4.py against the fp32 reference; gate is
absmax-rel < 2e-2, scheme measures relx ~ 2.2e-3, rely ~ 4.1e-3):
everything on-chip is fp16 (11-bit mantissa). fp16 matmuls run at full
PE rate on TRN2 (instruction_cost_v2.rs: cycles_per_row 1.0, same as
bf16), and the 8x finer mantissa vs bf16 kills the two error terms that
previously forced multi-pass bf16: G/C's rounding is a *structured*
perturbation that rides the ~33x x:y scale ratio through C, and xh's
representation error. Single-pass everywhere:
 - x conv: M single fp16 matmuls per 512-col tile into fp32 PSUM.
 - y = xh @ Ct + u @ Dt: 2 fp16 matmuls (D-term first: it only needs u,
   so the PE can start it while DVE casts xh).
 - u pre-cast to fp16 on host; x/y outputs written fp16, upcast on host.

Schedule: 8 mm per tile, 32 tiles (4 batch x 8 time) per core. The
y-phase of tile i is emitted after the x-phase of tile i+1 (one-stage
software pipeline) so the PE never waits on the PSUM->fp16 cast. Input
u rides the sync-engine DMA queue, weights + y-out ride the scalar
(Activation) HWDGE queue, x-out rides sync — two queues in parallel to
cut the cold-start serial latency.

Sharding: batch 32 -> 4 per core, 8 cores, SPMD, no collectives.
Layout: on-chip (d=128 partitions) x (time free dim); host pre-transposes
u and post-transposes y/x. The tiny x0 @ At^t boundary term (geometric
decay) is added on host for t < 64.
"""

import os
from functools import lru_cache

import numpy as np

B_FULL, T, D = 32, 4096, 128
N_CORES = 8
B_LOCAL = B_FULL // N_CORES  # 4

M_TAPS = int(os.environ.get("LTI_M", "5"))  # conv taps
GSPLIT = int(os.environ.get("LTI_GSPLIT", "0"))  # taps with hi/lo G split
M_X0 = 64  # host-side x0-term horizon; ||A^64|| ~ 3e-26
N_TILE = 512  # matmul free dim (one fp32 PSUM bank)

_last_result = None  # BassKernelResults of the most recent run (for test.py)


def _slots(m_taps, gsplit):
    """(slot_index, tap_m) pairs for the packed G tensor; hi/lo pairs
    for taps < gsplit, single hi slot after."""
    out = []
    w = 0
    for m in range(m_taps):
        out.append((w, m))
        w += 1
        if m < gsplit:
            out.append((w, m))  # lo part, same tap
            w += 1
    return out


def _host_matrices(S, K_raw):
    """Mirror reference._ssm_matrices bit-for-bit: fp32 jax on CPU."""
    import jax
    import jax.numpy as jnp

    cpu = jax.devices("cpu")[0]
    with jax.default_device(cpu):
        d_x = S.shape[0]
        sigma = jnp.maximum(jnp.linalg.norm(jnp.asarray(K_raw), ord=2), 1e-5)
        K = jnp.asarray(K_raw) / (sigma + 0.002)
        K11 = K[:d_x, :d_x]
        K12 = K[:d_x, d_x:]
        K21 = K[d_x:, :d_x]
        K22 = K[d_x:, d_x:]
        Sinv = jnp.linalg.inv(jnp.asarray(S))
        A = Sinv @ K11 @ jnp.asarray(S)
        Bm = Sinv @ K12  # GAMMA = 1.0
        C = K21 @ jnp.asarray(S)
        Dm = K22
        return (np.asarray(A), np.asarray(Bm), np.asarray(C), np.asarray(Dm))


@lru_cache(maxsize=4)
def _build(m_taps: int, gsplit: int):
    import concourse.mybir as mybir
    import concourse.tile as tile
    from concourse import bacc

    F32 = mybir.dt.float32
    F16 = mybir.dt.float16
    tp = T + m_taps
    n_tiles = T // N_TILE
    slots = _slots(m_taps, gsplit)
    nw = len(slots)

    nc = bacc.Bacc("TRN2", target_bir_lowering=False, num_devices=N_CORES)
    u_d = nc.dram_tensor("u", [B_LOCAL, D, tp], F16, kind="ExternalInput")
    g_d = nc.dram_tensor("g", [D, nw, D], F16, kind="ExternalInput")
    cd_d = nc.dram_tensor("cd", [D, 2, D], F16, kind="ExternalInput")
    y_d = nc.dram_tensor("y", [B_LOCAL, D, T], F16, kind="ExternalOutput")
    x_d = nc.dram_tensor("x", [B_LOCAL, D, T], F16, kind="ExternalOutput")

    GT = 2 * N_TILE  # a "group" is 2 tiles = one 2-bank PSUM tile
    n_grp = T // GT
    with tile.TileContext(nc) as tc:
        with (
            tc.tile_pool(name="const", bufs=1) as const,
            tc.tile_pool(name="upool", bufs=1) as upool,
            tc.tile_pool(name="xh", bufs=3) as xh_pool,
            tc.tile_pool(name="yh", bufs=2) as yh_pool,
            tc.tile_pool(name="px", bufs=2, space="PSUM") as px_pool,
            tc.tile_pool(name="py", bufs=2, space="PSUM") as py_pool,
        ):
            g_sb = const.tile([D, nw, D], F16)
            nc.scalar.dma_start(g_sb[:], g_d[:])
            cd_sb = const.tile([D, 2, D], F16)
            nc.scalar.dma_start(cd_sb[:], cd_d[:])

            # All of u is SBUF-resident (4 x 1.05MB fp16). Each batch is
            # loaded in 4 overlapping 1-group chunks, all issued up
            # front on the sync HWDGE queue, so chunk g of batch b
            # arrives well before its group and the first matmul only
            # waits for one ~0.26MB transfer. Casts and output DMAs run
            # at group (1024-col) granularity: vector/scalar/sync have a
            # large (~0.6us) fixed cost per instruction, so halving the
            # instruction count matters as much as the payload.
            CH = m_taps + GT  # chunk cols (one group + tap lookback)
            u_sbs = []
            for b in range(B_LOCAL):
                chunks = []
                for g in range(n_grp):
                    uc = upool.tile([D, CH], F16, tag=f"u{b}g{g}")
                    nc.sync.dma_start(uc[:], u_d[b][:, g * GT : g * GT + CH])
                    chunks.append(uc)
                u_sbs.append(chunks)

            pending = None  # (xh2, u_sb, b, t0) awaiting its y-phase

            def emit_y(item, split=False):
                xh2, u_sb, b, t0 = item
                py2 = py_pool.tile([D, 2, N_TILE], F32)
                for h in (0, 1):  # D-terms first: they only need u
                    s0 = m_taps + h * N_TILE
                    nc.tensor.matmul(
                        py2[:, h, :], cd_sb[:, 1, :], u_sb[:, s0 : s0 + N_TILE],
                        start=True, stop=False,
                    )
                if not split:
                    for h in (0, 1):
                        nc.tensor.matmul(
                            py2[:, h, :], cd_sb[:, 0, :], xh2[:, h, :],
                            start=False, stop=True,
                        )
                    yh2 = yh_pool.tile([D, 2, N_TILE], F16)
                    nc.vector.tensor_copy(yh2[:], py2[:])
                    nc.scalar.dma_start(y_d[b][:, t0 : t0 + GT], yh2[:])
                else:
                    # Tail drain: per-512 cast+DMA so the last transfers
                    # start (and finish) as early as possible.
                    yh2 = yh_pool.tile([D, 2, N_TILE], F16)
                    for h in (0, 1):
                        nc.tensor.matmul(
                            py2[:, h, :], cd_sb[:, 0, :], xh2[:, h, :],
                            start=False, stop=True,
                        )
                        nc.vector.tensor_copy(yh2[:, h, :], py2[:, h, :])
                        nc.scalar.dma_start(
                            y_d[b][:, t0 + h * N_TILE : t0 + (h + 1) * N_TILE],
                            yh2[:, h, :],
                        )

            for b in range(B_LOCAL):
                for g in range(n_grp):
                    last = b == B_LOCAL - 1 and g == n_grp - 1
                    u_sb = u_sbs[b][g]
                    t0 = g * GT
                    px2 = px_pool.tile([D, 2, N_TILE], F32)
                    xh2 = xh_pool.tile([D, 2, N_TILE], F16)
                    for h in (0, 1):
                        for k, (w, m) in enumerate(slots):
                            s = m_taps + h * N_TILE - 1 - m
                            nc.tensor.matmul(
                                px2[:, h, :], g_sb[:, w, :],
                                u_sb[:, s : s + N_TILE],
                                start=(k == 0), stop=(k == nw - 1),
                            )
                        if last:  # per-512 cast+DMA for the drain
                            nc.vector.tensor_copy(xh2[:, h, :], px2[:, h, :])
                            nc.sync.dma_start(
                                x_d[b][:, t0 + h * N_TILE : t0 + (h + 1) * N_TILE],
                                xh2[:, h, :],
                            )
                    if not last:
                        nc.vector.tensor_copy(xh2[:], px2[:])
                        # Alternate x-out between the two HWDGE rings: a
                        # single ring moves ~150GB/s and falls behind.
                        eng = nc.sync if (b * n_grp + g) % 2 else nc.scalar
                        eng.dma_start(x_d[b][:, t0 : t0 + GT], xh2[:])

                    if pending is not None:
                        emit_y(pending)
                    pending = (xh2, u_sb, b, t0)
            emit_y(pending, split=True)
    nc.compile()
    return nc


def _pack_inputs(u, x0, S, K_raw, m, gsplit):
    f16 = np.float16
    A, Bm, C, Dm = _host_matrices(S, K_raw)

    At = A.T.astype(np.float64)
    G = np.empty((m, D, D), dtype=np.float64)
    G[0] = Bm.T.astype(np.float64)
    for i in range(1, m):
        G[i] = G[i - 1] @ At

    slots = _slots(m, gsplit)
    nw = len(slots)
    gs = np.empty((nw, D, D), dtype=np.float32)
    seen = set()
    for w, tap in slots:
        g32 = G[tap].astype(np.float32)
        gh = g32.astype(f16).astype(np.float32)
        if tap not in seen:
            gs[w] = gh  # hi slot
            seen.add(tap)
        else:
            gs[w] = g32 - gh  # lo slot
    g_host = np.ascontiguousarray(gs.transpose(1, 0, 2)).astype(f16)

    # cd slots: (Ct, Dt) packed [d, 2, d].
    cd = np.stack([C.T.astype(np.float32), Dm.T.astype(np.float32)])
    cd_host = np.ascontiguousarray(cd.transpose(1, 0, 2)).astype(f16)

    in_maps = []
    for c in range(N_CORES):
        up = np.zeros((B_LOCAL, D, T + m), dtype=f16)
        for b in range(B_LOCAL):
            up[b, :, m:] = u[c * B_LOCAL + b].T.astype(f16)
        in_maps.append({"u": up, "g": g_host, "cd": cd_host})
    return in_maps, A, C


def kernel(u, x0, S, K_raw):
    global _last_result
    from concourse.bass_utils import run_bass_kernel_spmd

    m, gsplit = M_TAPS, GSPLIT
    u = np.asarray(u, dtype=np.float32)
    x0 = np.asarray(x0, dtype=np.float32)
    S = np.asarray(S, dtype=np.float32)
    K_raw = np.asarray(K_raw, dtype=np.float32)

    in_maps, A, C = _pack_inputs(u, x0, S, K_raw, m, gsplit)
    nc = _build(m, gsplit)
    res = run_bass_kernel_spmd(nc, in_maps, core_ids=list(range(N_CORES)))
    _last_result = res

    y_seq = np.empty((B_FULL, T, D), dtype=np.float32)
    x_seq = np.empty((B_FULL, T, D), dtype=np.float32)
    for c in range(N_CORES):
        ry = np.asarray(res.results[c]["y"], dtype=np.float32)
        rx = np.asarray(res.results[c]["x"], dtype=np.float32)
        for b in range(B_LOCAL):
            y_seq[c * B_LOCAL + b] = ry[b].T
            x_seq[c * B_LOCAL + b] = rx[b].T

    # x0 boundary term: x_t += x0 @ At^t, y_t += (x0 @ At^t) @ Ct, t < M_X0.
    At = A.T.astype(np.float64)
    Ct64 = C.T.astype(np.float64)
    xc = x0.astype(np.float64)
    for t in range(M_X0):
        x_seq[:, t, :] += xc.astype(np.float32)
        y_seq[:, t, :] += (xc @ Ct64).astype(np.float32)
        xc = xc @ At
    return (y_seq, x_seq)


# revision 12
# speedup vs baseline: 1.1821x; 1.1821x over previous
"""L2-bounded LTI cell (SSM scan) as a truncated convolution on TRN2.

Math: per batch b the reference computes
    x_{t+1} = x_t @ A.T + u_t @ B.T
    y_t     = x_t @ C.T + u_t @ D.T
with outputs x_seq[t] = x_t (pre-update state) and y_seq[t] = y_t.

K = K_raw / (||K_raw||_2 + 0.002) is a strict contraction, so
||A^m||_2 decays ~0.47x per step and the scan is a causal convolution
    x_t = x0 @ At^t + sum_{m<M} u_{t-1-m} @ G_m,   G_m = Bt @ At^m
truncated at M taps (M=6: structured trunc err ~2e-3, well under the
2e-2 gate).

Precision (validated in simacc4.py against the fp32 reference; gate is
absmax-rel < 2e-2, scheme measures relx ~ 2.2e-3, rely ~ 4.1e-3):
everything on-chip is fp16 (11-bit mantissa). fp16 matmuls run at full
PE rate on TRN2 (instruction_cost_v2.rs: cycles_per_row 1.0, same as
bf16), and the 8x finer mantissa vs bf16 kills the two error terms that
previously forced multi-pass bf16: G/C's rounding is a *structured*
perturbation that rides the ~33x x:y scale ratio through C, and xh's
representation error. Single-pass everywhere:
 - x conv: M single fp16 matmuls per 512-col tile into fp32 PSUM.
 - y = xh @ Ct + u @ Dt: 2 fp16 matmuls (D-term first: it only needs u,
   so the PE can start it while DVE casts xh).
 - u pre-cast to fp16 on host; x/y outputs written fp16, upcast on host.

Schedule: 8 mm per tile, 32 tiles (4 batch x 8 time) per core. The
y-phase of tile i is emitted after the x-phase of tile i+1 (one-stage
software pipeline) so the PE never waits on the PSUM->fp16 cast. Input
u rides the sync-engine DMA queue, weights + y-out ride the scalar
(Activation) HWDGE queue, x-out rides sync — two queues in parallel to
cut the cold-start serial latency.

Sharding: batch 32 -> 4 per core, 8 cores, SPMD, no collectives.
Layout: on-chip (d=128 partitions) x (time free dim); host pre-transposes
u and post-transposes y/x. The tiny x0 @ At^t boundary term (geometric
decay) is added on host for t < 64.
"""

import os
from functools import lru_cache

import numpy as np

B_FULL, T, D = 32, 4096, 128
N_CORES = 8
B_LOCAL = B_FULL // N_CORES  # 4

M_TAPS = int(os.environ.get("LTI_M", "5"))  # conv taps
GSPLIT = int(os.environ.get("LTI_GSPLIT", "0"))  # taps with hi/lo G split
M_X0 = 64  # host-side x0-term horizon; ||A^64|| ~ 3e-26
N_TILE = 512  # matmul free dim (one fp32 PSUM bank)

_last_result = None  # BassKernelResults of the most recent run (for test.py)


def _slots(m_taps, gsplit):
    """(slot_index, tap_m) pairs for the packed G tensor; hi/lo pairs
    for taps < gsplit, single hi slot after."""
    out = []
    w = 0
    for m in range(m_taps):
        out.append((w, m))
        w += 1
        if m < gsplit:
            out.append((w, m))  # lo part, same tap
            w += 1
    return out


def _host_matrices(S, K_raw):
    """Mirror reference._ssm_matrices bit-for-bit: fp32 jax on CPU."""
    import jax
    import jax.numpy as jnp

    cpu = jax.devices("cpu")[0]
    with jax.default_device(cpu):
        d_x = S.shape[0]
        sigma = jnp.maximum(jnp.linalg.norm(jnp.asarray(K_raw), ord=2), 1e-5)
        K = jnp.asarray(K_raw) / (sigma + 0.002)
        K11 = K[:d_x, :d_x]
        K12 = K[:d_x, d_x:]
        K21 = K[d_x:, :d_x]
        K22 = K[d_x:, d_x:]
        Sinv = jnp.linalg.inv(jnp.asarray(S))
        A = Sinv @ K11 @ jnp.asarray(S)
        Bm = Sinv @ K12  # GAMMA = 1.0
        C = K21 @ jnp.asarray(S)
        Dm = K22
        return (np.asarray(A), np.asarray(Bm), np.asarray(C), np.asarray(Dm))


@lru_cache(maxsize=4)
def _build(m_taps: int, gsplit: int):
    import concourse.mybir as mybir
    import concourse.tile as tile
    from concourse import bacc

    F32 = mybir.dt.float32
    F16 = mybir.dt.float16
    tp = T + m_taps
    n_tiles = T // N_TILE
    slots = _slots(m_taps, gsplit)
    nw = len(slots)

    nc = bacc.Bacc("TRN2", target_bir_lowering=False, num_devices=N_CORES)
    u_d = nc.dram_tensor("u", [B_LOCAL, D, tp], F16, kind="ExternalInput")
    g_d = nc.dram_tensor("g", [D, nw, D], F16, kind="ExternalInput")
    cd_d = nc.dram_tensor("cd", [D, 2, D], F16, kind="ExternalInput")
    y_d = nc.dram_tensor("y", [B_LOCAL, D, T], F16, kind="ExternalOutput")
    x_d = nc.dram_tensor("x", [B_LOCAL, D, T], F16, kind="ExternalOutput")

    GT = 2 * N_TILE  # a "group" is 2 tiles = one 2-bank PSUM tile
    n_grp = T // GT
    with tile.TileContext(nc) as tc:
        with (
            tc.tile_pool(name="const", bufs=1) as const,
            tc.tile_pool(name="upool", bufs=1) as upool,
            tc.tile_pool(name="xh", bufs=3) as xh_pool,
            tc.tile_pool(name="yh", bufs=2) as yh_pool,
            tc.tile_pool(name="px", bufs=2, space="PSUM") as px_pool,
            tc.tile_pool(name="py", bufs=2, space="PSUM") as py_pool,
        ):
            g_sb = const.tile([D, nw, D], F16)
            nc.scalar.dma_start(g_sb[:], g_d[:])
            cd_sb = const.tile([D, 2, D], F16)
            nc.scalar.dma_start(cd_sb[:], cd_d[:])

            # All of u is SBUF-resident (4 x 1.05MB fp16). Each batch is
            # loaded in 4 overlapping 1-group chunks, all issued up
            # front on the sync HWDGE queue, so chunk g of batch b
            # arrives well before its group and the first matmul only
            # waits for one ~0.26MB transfer. Casts and output DMAs run
            # at group (1024-col) granularity: vector/scalar/sync have a
            # large (~0.6us) fixed cost per instruction, so halving the
            # instruction count matters as much as the payload.
            CH = m_taps + GT  # chunk cols (one group + tap lookback)
            u_sbs = []
            for b in range(B_LOCAL):
                chunks = []
                for g in range(n_grp):
                    uc = upool.tile([D, CH], F16, tag=f"u{b}g{g}")
                    nc.sync.dma_start(uc[:], u_d[b][:, g * GT : g * GT + CH])
                    chunks.append(uc)
                u_sbs.append(chunks)

            pending = None  # (xh2, u_sb, b, t0) awaiting its y-phase

            def emit_y(item, split=False):
                xh2, u_sb, b, t0 = item
                py2 = py_pool.tile([D, 2, N_TILE], F32)
                for h in (0, 1):  # D-terms first: they only need u
                    s0 = m_taps + h * N_TILE
                    nc.tensor.matmul(
                        py2[:, h, :], cd_sb[:, 1, :], u_sb[:, s0 : s0 + N_TILE],
                        start=True, stop=False,
                    )
                if not split:
                    for h in (0, 1):
                        nc.tensor.matmul(
                            py2[:, h, :], cd_sb[:, 0, :], xh2[:, h, :],
                            start=False, stop=True,
                        )
                    yh2 = yh_pool.tile([D, 2, N_TILE], F16)
                    nc.vector.tensor_copy(yh2[:], py2[:])
                    nc.scalar.dma_start(y_d[b][:, t0 : t0 + GT], yh2[:])
                else:
                    # Tail drain: per-512 cast+DMA so the last transfers
                    # start (and finish) as early as possible.
                    yh2 = yh_pool.tile([D, 2, N_TILE], F16)
                    for h in (0, 1):
                        nc.tensor.matmul(
                            py2[:, h, :], cd_sb[:, 0, :], xh2[:, h, :],
                            start=False, stop=True,
                        )
                        nc.vector.tensor_copy(yh2[:, h, :], py2[:, h, :])
                        nc.scalar.dma_start(
                            y_d[b][:, t0 + h * N_TILE : t0 + (h + 1) * N_TILE],
                            yh2[:, h, :],
                        )

            for b in range(B_LOCAL):
                for g in range(n_grp):
                    last = b == B_LOCAL - 1 and g == n_grp - 1
                    u_sb = u_sbs[b][g]
                    t0 = g * GT
                    px2 = px_pool.tile([D, 2, N_TILE], F32)
                    xh2 = xh_pool.tile([D, 2, N_TILE], F16)
                    for h in (0, 1):
                        for k, (w, m) in enumerate(slots):
                            s = m_taps + h * N_TILE - 1 - m
                            nc.tensor.matmul(
                                px2[:, h, :], g_sb[:, w, :],
                                u_sb[:, s : s + N_TILE],
                                start=(k == 0), stop=(k == nw - 1),
                            )
                        if last:  # per-512 cast+DMA for the drain
                            nc.vector.tensor_copy(xh2[:, h, :], px2[:, h, :])
                            nc.sync.dma_start(
                                x_d[b][:, t0 + h * N_TILE : t0 + (h + 1) * N_TILE],
                                xh2[:, h, :],
                            )
                    if not last:
                        nc.vector.tensor_copy(xh2[:], px2[:])
                        nc.scalar.dma_start(x_d[b][:, t0 : t0 + GT], xh2[:])

                    if pending is not None:
                        emit_y(pending)
                    pending = (xh2, u_sb, b, t0)
            emit_y(pending, split=True)
    nc.compile()
    return nc


def _pack_inputs(u, x0, S, K_raw, m, gsplit):
    f16 = np.float16
    A, Bm, C, Dm = _host_matrices(S, K_raw)

    At = A.T.astype(np.float64)
    G = np.empty((m, D, D), dtype=np.float64)
    G[0] = Bm.T.astype(np.float64)
    for i in range(1, m):
        G[i] = G[i - 1] @ At

    slots = _slots(m, gsplit)
    nw = len(slots)
    gs = np.empty((nw, D, D), dtype=np.float32)
    seen = set()
    for w, tap in slots:
        g32 = G[tap].astype(np.float32)
        gh = g32.astype(f16).astype(np.float32)
        if tap not in seen:
            gs[w] = gh  # hi slot
            seen.add(tap)
        else:
            gs[w] = g32 - gh  # lo slot
    g_host = np.ascontiguousarray(gs.transpose(1, 0, 2)).astype(f16)

    # cd slots: (Ct, Dt) packed [d, 2, d].
    cd = np.stack([C.T.astype(np.float32), Dm.T.astype(np.float32)])
    cd_host = np.ascontiguousarray(cd.transpose(1, 0, 2)).astype(f16)

    in_maps = []
    for c in range(N_CORES):
        up = np.zeros((B_LOCAL, D, T + m), dtype=f16)
        for b in range(B_LOCAL):
            up[b, :, m:] = u[c * B_LOCAL + b].T.astype(f16)
        in_maps.append({"u": up, "g": g_host, "cd": cd_host})
    return in_maps, A, C


def kernel(u, x0, S, K_raw):
    global _last_result
    from concourse.bass_utils import run_bass_kernel_spmd

    m, gsplit = M_TAPS, GSPLIT
    u = np.asarray(u, dtype=np.float32)
    x0 = np.asarray(x0, dtype=np.float32)
    S = np.asarray(S, dtype=np.float32)
    K_raw = np.asarray(K_raw, dtype=np.float32)

    in_maps, A, C = _pack_inputs(u, x0, S, K_raw, m, gsplit)
    nc = _build(m, gsplit)
    res = run_bass_kernel_spmd(nc, in_maps, core_ids=list(range(N_CORES)))
    _last_result = res

    y_seq = np.empty((B_FULL, T, D), dtype=np.float32)
    x_seq = np.empty((B_FULL, T, D), dtype=np.float32)
    for c in range(N_CORES):
        ry = np.asarray(res.results[c]["y"], dtype=np.float32)
        rx = np.asarray(res.results[c]["x"], dtype=np.float32)
        for b in range(B_LOCAL):
            y_seq[c * B_LOCAL + b] = ry[b].T
            x_seq[c * B_LOCAL + b] = rx[b].T

    # x0 boundary term: x_t += x0 @ At^t, y_t += (x0 @ At^t) @ Ct, t < M_X0.
    At = A.T.astype(np.float64)
    Ct64 = C.T.astype(np.float64)
    xc = x0.astype(np.float64)
    for t in range(M_X0):
        x_seq[:, t, :] += xc.astype(np.float32)
        y_seq[:, t, :] += (xc @ Ct64).astype(np.float32)
        xc = xc @ At
    return (y_seq, x_seq)
